# revision 2
# baseline (speedup 1.0000x reference)
"""Trainium2 Bass kernel for the differentiable LogicLayer forward pass.

Math (per output neuron j with a = x[:, idx_a[j]], b = x[:, idx_b[j]]):
    w      = softmax(weights[j])          # [14]
    coeffs = w @ OP_COEFFS                # [4] -> c0, ca, cb, cab
    out[:, j] = c0 + ca*a + cb*b + cab*a*b

Sharding: data-parallel over batch across 8 NeuronCores (1024 rows each);
weights / indices replicated.  Per core the kernel works feature-major:
partition p of an SBUF tile holds output neuron j = t*128 + p, the free dim
holds the 1024-sample batch shard.  The gathers x[:, idx] become row
gathers of the transposed shard xT[4096, 1024], done with the SWDGE
dma_gather (16 SDMA engines, 4 KiB/descriptor).  The softmax+collapse of
the tiny weights runs on-device (ACT exp + DVE reductions).  Outputs are
written transposed ([16384, 1024] per core) and untransposed on the host.
"""

import sys

import numpy as np

try:  # the axon sitecustomize usually provides concourse already
    import concourse  # noqa: F401
except ImportError:  # pragma: no cover
    sys.path.insert(0, "/opt/trn_rl_repo")

import concourse.bacc as bacc
import concourse.mybir as mybir
import concourse.tile as tile
from concourse.bass_utils import run_bass_kernel_spmd
from concourse.library_config import mlp as mlp_library

F32 = mybir.dt.float32
I16 = mybir.dt.int16

NCORES = 8
BATCH, IN_DIM, OUT_DIM, NOPS = 8192, 4096, 16384, 14
B = BATCH // NCORES            # 1024 batch rows per core
NJC = 512                      # output neurons per gather chunk
NCH = OUT_DIM // NJC           # 32 chunks
SL = NJC // 128                # 4 partition-slices per chunk
NT = OUT_DIM // 128            # 128 coefficient columns

_OP_COEFFS = np.array([
    [0,  0,  0,  1],
    [0,  1,  0, -1],
    [0,  1,  0,  0],
    [0,  0,  1, -1],
    [0,  0,  1,  0],
    [0,  1,  1, -2],
    [0,  1,  1, -1],
    [1, -1, -1,  1],
    [1, -1, -1,  2],
    [1,  0, -1,  0],
    [1,  0, -1,  1],
    [1, -1,  0,  0],
    [1, -1,  0,  1],
    [1,  0,  0, -1],
], dtype=np.float32)


def build_program():
    """Build + compile the per-core Bass program (identical on all cores)."""
    nc = bacc.Bacc("TRN2", target_bir_lowering=False, debug=False,
                   num_devices=NCORES)

    xt = nc.dram_tensor("xt", [IN_DIM, B], F32, kind="ExternalInput")
    wre = nc.dram_tensor("wre", [128, NT, NOPS], F32, kind="ExternalInput")
    opc = nc.dram_tensor("opc", [128, 4, NT, NOPS], F32, kind="ExternalInput")
    idxa = nc.dram_tensor("idxa", [128, OUT_DIM // 16], I16, kind="ExternalInput")
    idxb = nc.dram_tensor("idxb", [128, OUT_DIM // 16], I16, kind="ExternalInput")
    out = nc.dram_tensor("out", [OUT_DIM, B], F32, kind="ExternalOutput")

    # out rows j = ci*NJC + s*128 + p  ->  [ci, p, s, b] view for stores
    out_r = out.ap().rearrange("(c s p) b -> c p s b", s=SL, p=128)

    mult = mybir.AluOpType.mult
    add = mybir.AluOpType.add
    ident = mybir.ActivationFunctionType.Identity
    expf = mybir.ActivationFunctionType.Exp

    with tile.TileContext(nc) as tc:
        nc.gpsimd.load_library(mlp_library)
        with (
            tc.tile_pool(name="const", bufs=1) as cpool,
            tc.tile_pool(name="coef", bufs=1) as kpool,
        ):
            ia_sb = cpool.tile([128, OUT_DIM // 16], I16)
            nc.sync.dma_start(ia_sb[:], idxa.ap())
            ib_sb = cpool.tile([128, OUT_DIM // 16], I16)
            nc.sync.dma_start(ib_sb[:], idxb.ap())

            # ---- coefficients: softmax over the 14 ops, collapsed to 4 ----
            with tc.tile_pool(name="init", bufs=1) as ipool:
                w_sb = ipool.tile([128, NT, NOPS], F32)
                nc.sync.dma_start(w_sb[:], wre.ap())
                opc_sb = ipool.tile([128, 4, NT, NOPS], F32)
                nc.sync.dma_start(opc_sb[:], opc.ap())

                e_sb = ipool.tile([128, NT, NOPS], F32)
                nc.scalar.activation(e_sb[:], w_sb[:], expf)
                ssum = ipool.tile([128, NT], F32)
                nc.vector.tensor_reduce(ssum[:], e_sb[:],
                                        mybir.AxisListType.X, add)
                rsum = ipool.tile([128, NT], F32)
                nc.vector.reciprocal(rsum[:], ssum[:])

                # coef[m]: [128, NT] with element (p, t) = coeff_m[t*128+p]
                coef = []
                for m in range(4):
                    tmp = ipool.tile([128, NT, NOPS], F32, tag="ctmp")
                    nc.vector.tensor_tensor(tmp[:], e_sb[:], opc_sb[:, m],
                                            op=mult)
                    cm = kpool.tile([128, NT], F32, tag=f"coef{m}")
                    nc.vector.tensor_reduce(cm[:], tmp[:],
                                            mybir.AxisListType.X, add)
                    nc.vector.tensor_tensor(cm[:], cm[:], rsum[:], op=mult)
                    coef.append(cm)
                c0, ca, cb, cab = coef

            # ---- main loop: gather a/b rows, combine, store ----
            with (
                tc.tile_pool(name="ga", bufs=3) as apool,
                tc.tile_pool(name="gb", bufs=3) as bpool,
                tc.tile_pool(name="go", bufs=2) as opool,
                tc.tile_pool(name="uv", bufs=4) as uvpool,
            ):
                w16 = NJC // 16  # idx columns per chunk
                for ci in range(NCH):
                    at = apool.tile([128, SL, B], F32)
                    bt = bpool.tile([128, SL, B], F32)
                    nc.gpsimd.dma_gather(
                        at[:], xt.ap(), ia_sb[:, ci * w16:(ci + 1) * w16],
                        NJC, NJC, B)
                    nc.gpsimd.dma_gather(
                        bt[:], xt.ap(), ib_sb[:, ci * w16:(ci + 1) * w16],
                        NJC, NJC, B)
                    ot = opool.tile([128, SL, B], F32)
                    for s in range(SL):
                        t = ci * SL + s
                        u = uvpool.tile([128, B], F32, tag="u")
                        v = uvpool.tile([128, B], F32, tag="v")
                        # u = cab*a + cb ; v = ca*a + c0  (per-partition s/b)
                        nc.scalar.activation(u[:], at[:, s], ident,
                                             bias=cb[:, t:t + 1],
                                             scale=cab[:, t:t + 1])
                        nc.scalar.activation(v[:], at[:, s], ident,
                                             bias=c0[:, t:t + 1],
                                             scale=ca[:, t:t + 1])
                        # out = u*b + v  (DVE)
                        nc.vector.tensor_tensor(u[:], u[:], bt[:, s], op=mult)
                        nc.vector.tensor_tensor(ot[:, s], u[:], v[:], op=add)
                    nc.sync.dma_start(out_r[ci], ot[:])

    nc.compile()
    return nc


_PROGRAM = None


def _get_program():
    global _PROGRAM
    if _PROGRAM is None:
        _PROGRAM = build_program()
    return _PROGRAM


def _wrap_idx(idx):
    """[OUT_DIM] int -> SWDGE-wrapped int16 [128, OUT_DIM//16].

    Per NJC-chunk c, columns [c*NJC//16:(c+1)*NJC//16] hold that chunk's
    indices with index i at (partition i%16, column i//16), replicated
    across the 8 groups of 16 partitions (one per Q7 core).
    """
    i16 = idx.astype(np.int16).reshape(NCH, NJC // 16, 16)
    w = i16.transpose(2, 0, 1).reshape(16, NCH * (NJC // 16))
    return np.ascontiguousarray(np.tile(w, (8, 1)))


def prepare_in_maps(x, weights, idx_a, idx_b):
    x = np.asarray(x, dtype=np.float32)
    weights = np.asarray(weights, dtype=np.float32)
    idx_a = np.asarray(idx_a)
    idx_b = np.asarray(idx_b)

    wre = np.ascontiguousarray(
        weights.reshape(NT, 128, NOPS).transpose(1, 0, 2))
    opc = np.ascontiguousarray(
        np.broadcast_to(_OP_COEFFS.T[None, :, None, :],
                        (128, 4, NT, NOPS))).astype(np.float32)
    ia = _wrap_idx(idx_a)
    ib = _wrap_idx(idx_b)

    in_maps = []
    for c in range(NCORES):
        xt = np.ascontiguousarray(x[c * B:(c + 1) * B].T)
        in_maps.append({"xt": xt, "wre": wre, "opc": opc,
                        "idxa": ia, "idxb": ib})
    return in_maps


def assemble_output(results):
    out = np.empty((BATCH, OUT_DIM), dtype=np.float32)
    for c in range(NCORES):
        out[c * B:(c + 1) * B] = results[c]["out"].T
    return out


def kernel(x, weights, idx_a, idx_b):
    nc = _get_program()
    in_maps = prepare_in_maps(x, weights, idx_a, idx_b)
    res = run_bass_kernel_spmd(nc, in_maps, list(range(NCORES)))
    return assemble_output(res.results)
